# revision 1
# baseline (speedup 1.0000x reference)
"""Trainium2 Bass kernel for nn_DecodePredictions (YOLO-style decode, B=16).

Strategy: pure data-parallel over batch (2 images per core x 8 cores).
Host-side: concat the 3 prediction levels into a flat [N_anchor, 85] tensor
per image, pad 8400 -> 8448 anchors so everything divides evenly, and lay
anchors out partition-blocked so every DMA moves large contiguous
per-partition chunks. The box fields are split host-side into (x,y) and
(w,h) planes so the whole box decode runs once up front on contiguous
layouts (single Exp -> one ACT table load); the per-tile ACT work is then a
single contiguous Sigmoid. Per-anchor grid/stride constants are precomputed
on host (input-independent) and streamed in as tiny aux tensors.

Device-side: box_all = decode(pa01, pa23, aux) once; then per tile of 6
anchors/partition: sigmoid logits, broadcast box into out lanes 0:4 (step-0
AP), scores into lane 5; the constant class-id lane 4 lives in persistent
output buffers written once at init. Output [128, 132, 80, 6] per core is
bit-identical in layout to the final [B, N*C, 6] rows owned by that core.
"""

import ml_dtypes
import numpy as np

N_CORES = 8
B = 16
B_PER_CORE = B // N_CORES  # 2
C = 80
F = 85
N_REAL = 8400              # 80*80 + 40*40 + 20*20
N_PAD = 8448               # = 66 * 128
P = 128
KPP = B_PER_CORE * N_PAD // P  # 132 anchors per partition
GK = 6                     # anchors per partition per tile
NT = KPP // GK             # 22 tiles
NOB = 6                    # persistent output buffers

_CACHE: dict = {}


def _build_nc():
    import concourse.bacc as bacc
    import concourse.tile as tile
    from concourse import mybir
    from contextlib import ExitStack

    nc = bacc.Bacc("TRN2", target_bir_lowering=False, debug=False)
    pa01 = nc.dram_tensor("pa01", [P, KPP, 2], mybir.dt.float32, kind="ExternalInput")
    pa23 = nc.dram_tensor("pa23", [P, KPP, 2], mybir.dt.float32, kind="ExternalInput")
    auxS = nc.dram_tensor("auxS", [P, KPP, 2], mybir.dt.float32, kind="ExternalInput")
    auxB = nc.dram_tensor("auxB", [P, KPP, 2], mybir.dt.float32, kind="ExternalInput")
    predsB = nc.dram_tensor("predsB", [P, KPP, 81], mybir.dt.bfloat16, kind="ExternalInput")
    clsc = nc.dram_tensor("clsc", [P, C], mybir.dt.float32, kind="ExternalInput")
    out = nc.dram_tensor("out", [P, KPP, C, 6], mybir.dt.float32, kind="ExternalOutput")

    fp32 = mybir.dt.float32
    AF = mybir.ActivationFunctionType

    with tile.TileContext(nc) as tc, ExitStack() as ctx:
        cpool = ctx.enter_context(tc.tile_pool(name="const", bufs=1))
        ipool = ctx.enter_context(tc.tile_pool(name="in", bufs=10))
        opool = ctx.enter_context(tc.tile_pool(name="out", bufs=1))
        tpool = ctx.enter_context(tc.tile_pool(name="tmp", bufs=6))

        pa23_t = cpool.tile([P, KPP, 2], fp32, tag="pa23")
        nc.sync.dma_start(out=pa23_t[:], in_=pa23[:])
        auxS_t = cpool.tile([P, KPP, 2], fp32, tag="auxS")
        nc.sync.dma_start(out=auxS_t[:], in_=auxS[:])
        pa01_t = cpool.tile([P, KPP, 2], fp32, tag="pa01")
        nc.sync.dma_start(out=pa01_t[:], in_=pa01[:])
        auxB_t = cpool.tile([P, KPP, 2], fp32, tag="auxB")
        nc.sync.dma_start(out=auxB_t[:], in_=auxB[:])
        cls_t = cpool.tile([P, C], fp32, tag="cls")
        nc.gpsimd.dma_start(out=cls_t[:], in_=clsc[:])

        # Whole-core box decode once, all on contiguous layouts:
        #   bb[:,0] = p_xy * s + grid*s ; bb[:,1] = bb[:,0] + exp(p_wh) * s
        # then interleave into box_t[p, k, 0:4] = (x1, y1, x2, y2).
        wh_t = cpool.tile([P, KPP, 2], fp32, tag="wh")
        nc.scalar.activation(wh_t[:], pa23_t[:], AF.Exp)
        nc.vector.tensor_mul(wh_t[:], wh_t[:], auxS_t[:])
        bb_t = cpool.tile([P, 2, KPP, 2], fp32, tag="bb")
        nc.vector.tensor_mul(bb_t[:, 0, :, :], pa01_t[:], auxS_t[:])
        nc.vector.tensor_add(bb_t[:, 0, :, :], bb_t[:, 0, :, :], auxB_t[:])
        nc.vector.tensor_add(bb_t[:, 1, :, :], bb_t[:, 0, :, :], wh_t[:])
        box_t = cpool.tile([P, KPP, 4], fp32, tag="box")
        nc.vector.tensor_copy(
            box_t[:].rearrange("p k (jh jl) -> p k jh jl", jh=2),
            bb_t[:].rearrange("p jh k jl -> p k jh jl"),
        )

        # Persistent out buffers; constant class-id lane written once (on the
        # Scalar engine, which is otherwise idle, to keep GpSimd off DVE's
        # SBUF ports during the box-decode chain).
        ot_bufs = [
            opool.tile([P, GK, C, 6], fp32, tag=f"ot{j}", name=f"ot{j}")
            for j in range(NOB)
        ]
        for j in range(NOB):
            nc.scalar.copy(
                ot_bufs[j][:, :, :, 4:5],
                cls_t[:].unsqueeze(1).unsqueeze(3).broadcast_to([P, GK, C, 1]),
            )

        pt2 = None
        for t in range(NT):
            sl = slice(t * GK, (t + 1) * GK)
            if t % 2 == 0:
                # One input DMA feeds two compute tiles: per-partition chunks
                # double to 3888 B, halving descriptor + issue overhead.
                pt2 = ipool.tile([P, 2 * GK, 81], mybir.dt.bfloat16, tag="pt", name=f"pt{t}")
                # First few input tiles ride the fast HWDGE queue so the DMA
                # engines have bytes to move during the startup window.
                dma_eng = nc.sync if t < 6 else nc.gpsimd
                dma_eng.dma_start(
                    out=pt2[:], in_=predsB[:, t * GK : (t + 2) * GK, :]
                )

            sig = tpool.tile([P, GK, 81], fp32, tag="sig")
            half = t % 2
            nc.scalar.activation(
                sig[:], pt2[:, half * GK : (half + 1) * GK, :], AF.Sigmoid
            )

            ot = ot_bufs[t % NOB]
            nc.vector.tensor_copy(
                ot[:, :, :, 0:4],
                box_t[:, sl, :].unsqueeze(2).broadcast_to([P, GK, C, 4]),
            )
            nc.vector.tensor_mul(
                ot[:, :, :, 5:6],
                sig[:, :, 1:81].unsqueeze(3),
                sig[:, :, 0:1].broadcast_to([P, GK, C]).unsqueeze(3),
            )

            nc.sync.dma_start(out=out[:, sl, :, :], in_=ot[:])

    nc.compile()
    return nc


def _host_consts():
    # Per-anchor (stride, stride) and (gx*stride, gy*stride), padded to N_PAD.
    s = np.ones(N_PAD, np.float32)
    bx = np.zeros(N_PAD, np.float32)
    by = np.zeros(N_PAD, np.float32)
    off = 0
    for g, st in ((80, 8.0), (40, 16.0), (20, 32.0)):
        n = g * g
        i = np.arange(n)
        s[off : off + n] = st
        bx[off : off + n] = (i % g) * st
        by[off : off + n] = (i // g) * st
        off += n
    auxS = np.stack([s, s], axis=-1).astype(np.float32)
    auxB = np.stack([bx, by], axis=-1).astype(np.float32)
    auxS = np.concatenate([auxS] * B_PER_CORE, 0).reshape(P, KPP, 2)
    auxB = np.concatenate([auxB] * B_PER_CORE, 0).reshape(P, KPP, 2)
    cls = np.broadcast_to(np.arange(C, dtype=np.float32), (P, C)).copy()
    return np.ascontiguousarray(auxS), np.ascontiguousarray(auxB), cls


def _host_in_maps(pred0, pred1, pred2):
    auxS, auxB, cls = _CACHE["consts"]
    pred0 = np.asarray(pred0, np.float32).reshape(B, -1, F)
    pred1 = np.asarray(pred1, np.float32).reshape(B, -1, F)
    pred2 = np.asarray(pred2, np.float32).reshape(B, -1, F)
    in_maps = []
    for core in range(N_CORES):
        flat = np.zeros((B_PER_CORE * N_PAD, F), np.float32)
        for j in range(B_PER_CORE):
            b = core * B_PER_CORE + j
            flat[j * N_PAD : j * N_PAD + N_REAL] = np.concatenate(
                [pred0[b], pred1[b], pred2[b]], axis=0
            )
        in_maps.append(
            {
                "pa01": np.ascontiguousarray(flat[:, 0:2]).reshape(P, KPP, 2),
                "pa23": np.ascontiguousarray(flat[:, 2:4]).reshape(P, KPP, 2),
                "auxS": auxS,
                "auxB": auxB,
                "predsB": np.ascontiguousarray(flat[:, 4:85]).astype(ml_dtypes.bfloat16).reshape(P, KPP, 81),
                "clsc": cls,
            }
        )
    return in_maps


def kernel(images, pred0, pred1, pred2):
    from concourse.bass_utils import run_bass_kernel_spmd

    if "nc" not in _CACHE:
        _CACHE["consts"] = _host_consts()
        _CACHE["nc"] = _build_nc()
    nc = _CACHE["nc"]

    in_maps = _host_in_maps(pred0, pred1, pred2)
    res = run_bass_kernel_spmd(nc, in_maps, list(range(N_CORES)))
    outs = [
        r["out"].reshape(B_PER_CORE, N_PAD * C, 6)[:, : N_REAL * C, :]
        for r in res.results
    ]
    return np.concatenate(outs, axis=0)



# revision 2
# speedup vs baseline: 1.3362x; 1.3362x over previous
"""Trainium2 Bass kernel for nn_DecodePredictions (YOLO-style decode, B=16).

Strategy: pure data-parallel over batch (2 images per core x 8 cores).
Host-side: concat the 3 prediction levels into a flat [N_anchor, 85] tensor
per image, pad 8400 -> 8448 anchors so everything divides evenly, and lay
anchors out partition-blocked so every DMA moves large contiguous
per-partition chunks. The box fields are split host-side into (x,y) and
(w,h) planes so the whole box decode runs once up front on contiguous
layouts (single Exp -> one ACT table load); the per-tile ACT work is then a
single contiguous Sigmoid. Per-anchor grid/stride constants are precomputed
on host (input-independent) and streamed in as tiny aux tensors.

The whole output path is bf16: the gate is rel_err < 2e-2 against
absmax ~1958, and bf16 rounding of the box coords costs at most
ULP(2048)/2 = 4 absolute (~2e-3 relative), so halving the dominant
HBM write traffic is free accuracy-wise. The host upconverts to fp32
while scattering per-core results into the final array.

Device-side: box_all = decode(pa01, pa23, aux) once in fp32, converted to
bf16; then per tile of 11 anchors/partition: sigmoid logits (bf16), DVE
broadcast-copies box into out lanes 0:4 and the obj*cls product into lane
5; the constant class-id lane 4 lives in persistent output buffers written
once at init. Output [128, 132, 80, 6] bf16 per core matches the final
[B, N*C, 6] rows owned by that core.
"""

import ml_dtypes
import numpy as np

N_CORES = 8
B = 16
B_PER_CORE = B // N_CORES  # 2
C = 80
F = 85
N_REAL = 8400              # 80*80 + 40*40 + 20*20
N_PAD = 8448               # = 66 * 128
P = 128
KPP = B_PER_CORE * N_PAD // P  # 132 anchors per partition
GK = 11                    # anchors per partition per tile
NT = KPP // GK             # 12 tiles
NOB = 4                    # persistent output buffers
ICH = 33                   # anchors per input-chunk DMA (3 tiles)
NIC = KPP // ICH           # 4 input chunks

_CACHE: dict = {}


def _build_nc():
    import concourse.bacc as bacc
    import concourse.tile as tile
    from concourse import mybir
    from contextlib import ExitStack

    nc = bacc.Bacc("TRN2", target_bir_lowering=False, debug=False)
    pa01 = nc.dram_tensor("pa01", [P, KPP, 2], mybir.dt.float32, kind="ExternalInput")
    pa23 = nc.dram_tensor("pa23", [P, KPP, 2], mybir.dt.float32, kind="ExternalInput")
    auxS = nc.dram_tensor("auxS", [P, KPP, 2], mybir.dt.float32, kind="ExternalInput")
    auxB = nc.dram_tensor("auxB", [P, KPP, 2], mybir.dt.float32, kind="ExternalInput")
    predsB = nc.dram_tensor("predsB", [P, KPP, 81], mybir.dt.bfloat16, kind="ExternalInput")
    clsc = nc.dram_tensor("clsc", [P, C], mybir.dt.bfloat16, kind="ExternalInput")
    out = nc.dram_tensor("out", [P, KPP, C, 6], mybir.dt.bfloat16, kind="ExternalOutput")

    fp32 = mybir.dt.float32
    bf16 = mybir.dt.bfloat16
    AF = mybir.ActivationFunctionType

    with tile.TileContext(nc) as tc, ExitStack() as ctx:
        cpool = ctx.enter_context(tc.tile_pool(name="const", bufs=1))
        ipool = ctx.enter_context(tc.tile_pool(name="in", bufs=NIC))
        opool = ctx.enter_context(tc.tile_pool(name="out", bufs=1))
        tpool = ctx.enter_context(tc.tile_pool(name="tmp", bufs=4))

        pa23_t = cpool.tile([P, KPP, 2], fp32, tag="pa23")
        nc.sync.dma_start(out=pa23_t[:], in_=pa23[:])
        auxS_t = cpool.tile([P, KPP, 2], fp32, tag="auxS")
        nc.sync.dma_start(out=auxS_t[:], in_=auxS[:])
        pa01_t = cpool.tile([P, KPP, 2], fp32, tag="pa01")
        nc.sync.dma_start(out=pa01_t[:], in_=pa01[:])
        auxB_t = cpool.tile([P, KPP, 2], fp32, tag="auxB")
        nc.sync.dma_start(out=auxB_t[:], in_=auxB[:])
        cls_t = cpool.tile([P, C], bf16, tag="cls")
        nc.gpsimd.dma_start(out=cls_t[:], in_=clsc[:])

        # Input chunks: 33 anchors (3 compute tiles) per DMA, ~684 KB each.
        # Chunk 0 rides the fast HWDGE queue so the first sigmoid isn't
        # gated on SWDGE's ~1us first-byte latency.
        in_tiles = []
        for ci in range(NIC):
            it = ipool.tile([P, ICH, 81], bf16, tag="pt", name=f"pt{ci}")
            dma_eng = nc.sync if ci == 0 else nc.gpsimd
            dma_eng.dma_start(out=it[:], in_=predsB[:, ci * ICH : (ci + 1) * ICH, :])
            in_tiles.append(it)

        # Whole-core box decode once, all on contiguous layouts:
        #   bb[:,0] = p_xy * s + grid*s ; bb[:,1] = bb[:,0] + exp(p_wh) * s
        # then interleave into box_t[p, k, 0:4] = (x1, y1, x2, y2) as bf16.
        wh_t = cpool.tile([P, KPP, 2], fp32, tag="wh")
        nc.scalar.activation(wh_t[:], pa23_t[:], AF.Exp)
        nc.vector.tensor_mul(wh_t[:], wh_t[:], auxS_t[:])
        bb_t = cpool.tile([P, 2, KPP, 2], fp32, tag="bb")
        nc.vector.tensor_mul(bb_t[:, 0, :, :], pa01_t[:], auxS_t[:])
        nc.vector.tensor_add(bb_t[:, 0, :, :], bb_t[:, 0, :, :], auxB_t[:])
        nc.vector.tensor_add(bb_t[:, 1, :, :], bb_t[:, 0, :, :], wh_t[:])
        box_t = cpool.tile([P, KPP, 4], bf16, tag="box")
        nc.vector.tensor_copy(
            box_t[:].rearrange("p k (jh jl) -> p k jh jl", jh=2),
            bb_t[:].rearrange("p jh k jl -> p k jh jl"),
        )

        # Persistent out buffers; constant class-id lane written once (on the
        # Scalar engine, which is otherwise idle, to keep GpSimd off DVE's
        # SBUF ports during the box-decode chain).
        ot_bufs = [
            opool.tile([P, GK, C, 6], bf16, tag=f"ot{j}", name=f"ot{j}")
            for j in range(NOB)
        ]
        for j in range(NOB):
            nc.scalar.copy(
                ot_bufs[j][:, :, :, 4:5],
                cls_t[:].unsqueeze(1).unsqueeze(3).broadcast_to([P, GK, C, 1]),
            )

        for t in range(NT):
            sl = slice(t * GK, (t + 1) * GK)
            src = in_tiles[t // 3]
            ksl = slice((t % 3) * GK, (t % 3 + 1) * GK)

            sig = tpool.tile([P, GK, 81], bf16, tag="sig")
            nc.scalar.activation(sig[:], src[:, ksl, :], AF.Sigmoid)

            ot = ot_bufs[t % NOB]
            nc.vector.tensor_copy(
                ot[:, :, :, 0:4],
                box_t[:, sl, :].unsqueeze(2).broadcast_to([P, GK, C, 4]),
            )
            nc.vector.tensor_mul(
                ot[:, :, :, 5:6],
                sig[:, :, 1:81].unsqueeze(3),
                sig[:, :, 0:1].broadcast_to([P, GK, C]).unsqueeze(3),
            )

            nc.sync.dma_start(out=out[:, sl, :, :], in_=ot[:])

    nc.compile()
    return nc


def _host_consts():
    # Per-anchor (stride, stride) and (gx*stride, gy*stride), padded to N_PAD.
    s = np.ones(N_PAD, np.float32)
    bx = np.zeros(N_PAD, np.float32)
    by = np.zeros(N_PAD, np.float32)
    off = 0
    for g, st in ((80, 8.0), (40, 16.0), (20, 32.0)):
        n = g * g
        i = np.arange(n)
        s[off : off + n] = st
        bx[off : off + n] = (i % g) * st
        by[off : off + n] = (i // g) * st
        off += n
    auxS = np.stack([s, s], axis=-1).astype(np.float32)
    auxB = np.stack([bx, by], axis=-1).astype(np.float32)
    auxS = np.concatenate([auxS] * B_PER_CORE, 0).reshape(P, KPP, 2)
    auxB = np.concatenate([auxB] * B_PER_CORE, 0).reshape(P, KPP, 2)
    cls = np.broadcast_to(
        np.arange(C, dtype=np.float32).astype(ml_dtypes.bfloat16), (P, C)
    ).copy()
    return np.ascontiguousarray(auxS), np.ascontiguousarray(auxB), cls


def _host_in_maps(pred0, pred1, pred2):
    auxS, auxB, cls = _CACHE["consts"]
    pred0 = np.asarray(pred0, np.float32).reshape(B, -1, F)
    pred1 = np.asarray(pred1, np.float32).reshape(B, -1, F)
    pred2 = np.asarray(pred2, np.float32).reshape(B, -1, F)
    in_maps = []
    for core in range(N_CORES):
        flat = np.zeros((B_PER_CORE * N_PAD, F), np.float32)
        for j in range(B_PER_CORE):
            b = core * B_PER_CORE + j
            flat[j * N_PAD : j * N_PAD + N_REAL] = np.concatenate(
                [pred0[b], pred1[b], pred2[b]], axis=0
            )
        in_maps.append(
            {
                "pa01": np.ascontiguousarray(flat[:, 0:2]).reshape(P, KPP, 2),
                "pa23": np.ascontiguousarray(flat[:, 2:4]).reshape(P, KPP, 2),
                "auxS": auxS,
                "auxB": auxB,
                "predsB": np.ascontiguousarray(flat[:, 4:85]).astype(ml_dtypes.bfloat16).reshape(P, KPP, 81),
                "clsc": cls,
            }
        )
    return in_maps


def kernel(images, pred0, pred1, pred2):
    from concourse.bass_utils import run_bass_kernel_spmd

    if "nc" not in _CACHE:
        _CACHE["consts"] = _host_consts()
        _CACHE["nc"] = _build_nc()
    nc = _CACHE["nc"]

    in_maps = _host_in_maps(pred0, pred1, pred2)
    res = run_bass_kernel_spmd(nc, in_maps, list(range(N_CORES)))
    final = np.empty((B, N_REAL * C, 6), np.float32)
    for core, r in enumerate(res.results):
        final[core * B_PER_CORE : (core + 1) * B_PER_CORE] = r["out"].reshape(
            B_PER_CORE, N_PAD * C, 6
        )[:, : N_REAL * C, :]
    return final


# revision 5
# speedup vs baseline: 1.3834x; 1.0353x over previous
"""Trainium2 Bass kernel for nn_DecodePredictions (YOLO-style decode, B=16).

Strategy: pure data-parallel over batch (2 images per core x 8 cores).
Host-side: concat the 3 prediction levels into a flat [N_anchor, 85] tensor
per image, pad 8400 -> 8448 anchors so everything divides evenly, and lay
anchors out partition-blocked so every DMA moves large contiguous
per-partition chunks. The box fields are split host-side into (x,y) and
(w,h) planes so the whole box decode runs once up front on contiguous
layouts (single Exp -> one ACT table load); the per-tile ACT work is then a
single contiguous Sigmoid. Per-anchor grid/stride constants are precomputed
on host (input-independent) and streamed in as tiny aux tensors.

The whole output path is bf16: the gate is rel_err < 2e-2 against
absmax ~1958, and bf16 rounding of the box coords costs at most
ULP(2048)/2 = 4 absolute (~2e-3 relative), so halving the dominant
HBM write traffic is free accuracy-wise. The host upconverts to fp32
while scattering per-core results into the final array.

Device-side: box_all = decode(pa01, pa23, aux) once in fp32, converted to
bf16; then per tile of 11 anchors/partition: sigmoid logits (bf16), DVE
broadcast-copies box into out lanes 0:4 and the obj*cls product into lane
5; the constant class-id lane 4 lives in persistent output buffers written
once at init. Output [128, 132, 80, 6] bf16 per core matches the final
[B, N*C, 6] rows owned by that core.
"""

import ml_dtypes
import numpy as np

N_CORES = 8
B = 16
B_PER_CORE = B // N_CORES  # 2
C = 80
F = 85
N_REAL = 8400              # 80*80 + 40*40 + 20*20
N_PAD = 8448               # = 66 * 128
P = 128
KPP = B_PER_CORE * N_PAD // P  # 132 anchors per partition
GK = 11                    # anchors per partition per tile
NT = KPP // GK             # 12 tiles
NOB = 4                    # persistent output buffers
ICH = 33                   # anchors per input-chunk DMA (3 tiles)
NIC = KPP // ICH           # 4 input chunks

_CACHE: dict = {}


def _build_nc():
    import concourse.bacc as bacc
    import concourse.tile as tile
    from concourse import mybir
    from contextlib import ExitStack

    nc = bacc.Bacc("TRN2", target_bir_lowering=False, debug=False)
    pa01 = nc.dram_tensor("pa01", [P, KPP, 2], mybir.dt.float32, kind="ExternalInput")
    pa23 = nc.dram_tensor("pa23", [P, KPP, 2], mybir.dt.float32, kind="ExternalInput")
    auxS = nc.dram_tensor("auxS", [P, KPP, 2], mybir.dt.float32, kind="ExternalInput")
    auxB = nc.dram_tensor("auxB", [P, KPP, 2], mybir.dt.float32, kind="ExternalInput")
    predsB = nc.dram_tensor("predsB", [P, KPP, 81], mybir.dt.bfloat16, kind="ExternalInput")
    clsc = nc.dram_tensor("clsc", [P, C], mybir.dt.bfloat16, kind="ExternalInput")
    out = nc.dram_tensor("out", [P, KPP, C, 6], mybir.dt.bfloat16, kind="ExternalOutput")

    fp32 = mybir.dt.float32
    bf16 = mybir.dt.bfloat16
    AF = mybir.ActivationFunctionType

    with tile.TileContext(nc) as tc, ExitStack() as ctx:
        cpool = ctx.enter_context(tc.tile_pool(name="const", bufs=1))
        ipool = ctx.enter_context(tc.tile_pool(name="in", bufs=NIC))
        opool = ctx.enter_context(tc.tile_pool(name="out", bufs=1))
        tpool = ctx.enter_context(tc.tile_pool(name="tmp", bufs=4))

        pa23_t = cpool.tile([P, KPP, 2], fp32, tag="pa23")
        nc.sync.dma_start(out=pa23_t[:], in_=pa23[:])
        auxS_t = cpool.tile([P, KPP, 2], fp32, tag="auxS")
        nc.sync.dma_start(out=auxS_t[:], in_=auxS[:])
        pa01_t = cpool.tile([P, KPP, 2], fp32, tag="pa01")
        nc.sync.dma_start(out=pa01_t[:], in_=pa01[:])
        auxB_t = cpool.tile([P, KPP, 2], fp32, tag="auxB")
        nc.sync.dma_start(out=auxB_t[:], in_=auxB[:])
        cls_t = cpool.tile([P, C], bf16, tag="cls")
        nc.sync.dma_start(out=cls_t[:], in_=clsc[:])

        # Input chunks: 33 anchors (3 compute tiles) per DMA, ~684 KB each.
        # Chunk 0 rides the Sync HWDGE ring ahead of the big output writes;
        # the rest go on the second HWDGE ring (ACT) so neither GpSimd (now
        # doing the score muls) nor the output FIFO is in their path.
        in_tiles = []
        for ci in range(NIC):
            it = ipool.tile([P, ICH, 81], bf16, tag="pt", name=f"pt{ci}")
            dma_eng = nc.sync if ci == 0 else nc.scalar
            dma_eng.dma_start(out=it[:], in_=predsB[:, ci * ICH : (ci + 1) * ICH, :])
            in_tiles.append(it)

        # Whole-core box decode once, all on contiguous layouts:
        #   bb[:,0] = p_xy * s + grid*s ; bb[:,1] = bb[:,0] + exp(p_wh) * s
        # then interleave into box_t[p, k, 0:4] = (x1, y1, x2, y2) as bf16.
        wh_t = cpool.tile([P, KPP, 2], fp32, tag="wh")
        nc.scalar.activation(wh_t[:], pa23_t[:], AF.Exp)
        nc.vector.tensor_mul(wh_t[:], wh_t[:], auxS_t[:])
        bb_t = cpool.tile([P, 2, KPP, 2], fp32, tag="bb")
        nc.vector.tensor_mul(bb_t[:, 0, :, :], pa01_t[:], auxS_t[:])
        nc.vector.tensor_add(bb_t[:, 0, :, :], bb_t[:, 0, :, :], auxB_t[:])
        nc.vector.tensor_add(bb_t[:, 1, :, :], bb_t[:, 0, :, :], wh_t[:])
        box_t = cpool.tile([P, KPP, 4], bf16, tag="box")
        nc.vector.tensor_copy(
            box_t[:].rearrange("p k (jh jl) -> p k jh jl", jh=2),
            bb_t[:].rearrange("p jh k jl -> p k jh jl"),
        )

        # Persistent out buffers; constant class-id lane written once (on the
        # Scalar engine, which is otherwise idle, to keep GpSimd off DVE's
        # SBUF ports during the box-decode chain).
        ot_bufs = [
            opool.tile([P, GK, C, 6], bf16, tag=f"ot{j}", name=f"ot{j}")
            for j in range(NOB)
        ]
        for j in range(NOB):
            nc.scalar.copy(
                ot_bufs[j][:, :, :, 4:5],
                cls_t[:].unsqueeze(1).unsqueeze(3).broadcast_to([P, GK, C, 1]),
            )

        # One sigmoid per input chunk (3 tiles) to amortize ACT's ~352-cycle
        # fixed overhead per activation.
        sig_tiles = []
        for ci in range(NIC):
            sig = tpool.tile([P, ICH, 81], bf16, tag="sig", name=f"sig{ci}")
            nc.scalar.activation(sig[:], in_tiles[ci][:], AF.Sigmoid)
            sig_tiles.append(sig)

        for t in range(NT):
            sl = slice(t * GK, (t + 1) * GK)
            sig = sig_tiles[t // 3]
            ksl = slice((t % 3) * GK, (t % 3 + 1) * GK)

            ot = ot_bufs[t % NOB]
            nc.vector.tensor_copy(
                ot[:, :, :, 0:4],
                box_t[:, sl, :].unsqueeze(2).broadcast_to([P, GK, C, 4]),
            )
            # A/B probe: even tiles run the score mul on DVE with squeezed
            # (3-dim) APs, odd tiles on GpSimd — trace shows which is faster.
            mul_eng = nc.vector if t % 2 == 0 else nc.gpsimd
            mul_eng.tensor_mul(
                ot[:, :, :, 5],
                sig[:, ksl, 1:81],
                sig[:, ksl, 0:1].broadcast_to([P, GK, C]),
            )

            nc.sync.dma_start(out=out[:, sl, :, :], in_=ot[:])

    nc.compile()
    return nc


def _host_consts():
    # Per-anchor (stride, stride) and (gx*stride, gy*stride), padded to N_PAD.
    s = np.ones(N_PAD, np.float32)
    bx = np.zeros(N_PAD, np.float32)
    by = np.zeros(N_PAD, np.float32)
    off = 0
    for g, st in ((80, 8.0), (40, 16.0), (20, 32.0)):
        n = g * g
        i = np.arange(n)
        s[off : off + n] = st
        bx[off : off + n] = (i % g) * st
        by[off : off + n] = (i // g) * st
        off += n
    auxS = np.stack([s, s], axis=-1).astype(np.float32)
    auxB = np.stack([bx, by], axis=-1).astype(np.float32)
    auxS = np.concatenate([auxS] * B_PER_CORE, 0).reshape(P, KPP, 2)
    auxB = np.concatenate([auxB] * B_PER_CORE, 0).reshape(P, KPP, 2)
    cls = np.broadcast_to(
        np.arange(C, dtype=np.float32).astype(ml_dtypes.bfloat16), (P, C)
    ).copy()
    return np.ascontiguousarray(auxS), np.ascontiguousarray(auxB), cls


def _host_in_maps(pred0, pred1, pred2):
    auxS, auxB, cls = _CACHE["consts"]
    pred0 = np.asarray(pred0, np.float32).reshape(B, -1, F)
    pred1 = np.asarray(pred1, np.float32).reshape(B, -1, F)
    pred2 = np.asarray(pred2, np.float32).reshape(B, -1, F)
    in_maps = []
    for core in range(N_CORES):
        flat = np.zeros((B_PER_CORE * N_PAD, F), np.float32)
        for j in range(B_PER_CORE):
            b = core * B_PER_CORE + j
            flat[j * N_PAD : j * N_PAD + N_REAL] = np.concatenate(
                [pred0[b], pred1[b], pred2[b]], axis=0
            )
        in_maps.append(
            {
                "pa01": np.ascontiguousarray(flat[:, 0:2]).reshape(P, KPP, 2),
                "pa23": np.ascontiguousarray(flat[:, 2:4]).reshape(P, KPP, 2),
                "auxS": auxS,
                "auxB": auxB,
                "predsB": np.ascontiguousarray(flat[:, 4:85]).astype(ml_dtypes.bfloat16).reshape(P, KPP, 81),
                "clsc": cls,
            }
        )
    return in_maps


def kernel(images, pred0, pred1, pred2):
    from concourse.bass_utils import run_bass_kernel_spmd

    if "nc" not in _CACHE:
        _CACHE["consts"] = _host_consts()
        _CACHE["nc"] = _build_nc()
    nc = _CACHE["nc"]

    in_maps = _host_in_maps(pred0, pred1, pred2)
    res = run_bass_kernel_spmd(nc, in_maps, list(range(N_CORES)))
    final = np.empty((B, N_REAL * C, 6), np.float32)
    for core, r in enumerate(res.results):
        final[core * B_PER_CORE : (core + 1) * B_PER_CORE] = r["out"].reshape(
            B_PER_CORE, N_PAD * C, 6
        )[:, : N_REAL * C, :]
    return final


# revision 8
# speedup vs baseline: 1.3964x; 1.0094x over previous
"""Trainium2 Bass kernel for nn_DecodePredictions (YOLO-style decode, B=16).

Strategy: pure data-parallel over batch (2 images per core x 8 cores).
Host-side: concat the 3 prediction levels into a flat [N_anchor, 85] tensor
per image, pad 8400 -> 8448 anchors so everything divides evenly, and lay
anchors out partition-blocked so every DMA moves large contiguous
per-partition chunks. The box fields are split host-side into (x,y) and
(w,h) planes so the whole box decode runs once up front on contiguous
layouts (single Exp -> one ACT table load); the per-tile ACT work is then a
single contiguous Sigmoid. Per-anchor grid/stride constants are precomputed
on host (input-independent) and streamed in as tiny aux tensors.

The whole output path is bf16: the gate is rel_err < 2e-2 against
absmax ~1958, and bf16 rounding of the box coords costs at most
ULP(2048)/2 = 4 absolute (~2e-3 relative), so halving the dominant
HBM write traffic is free accuracy-wise. The host upconverts to fp32
while scattering per-core results into the final array.

Device-side: box_all = decode(pa01, pa23, aux) once in fp32, converted to
bf16; then per tile of 11 anchors/partition: sigmoid logits (bf16), DVE
broadcast-copies box into out lanes 0:4 and the obj*cls product into lane
5; the constant class-id lane 4 lives in persistent output buffers written
once at init. Output [128, 132, 80, 6] bf16 per core matches the final
[B, N*C, 6] rows owned by that core.
"""

import ml_dtypes
import numpy as np

N_CORES = 8
B = 16
B_PER_CORE = B // N_CORES  # 2
C = 80
F = 85
N_REAL = 8400              # 80*80 + 40*40 + 20*20
N_PAD = 8448               # = 66 * 128
P = 128
KPP = B_PER_CORE * N_PAD // P  # 132 anchors per partition
GK = 11                    # anchors per partition per tile
NT = KPP // GK             # 12 tiles
NOB = 4                    # persistent output buffers
ICH = 33                   # anchors per input-chunk DMA (3 tiles)
NIC = KPP // ICH           # 4 input chunks

_CACHE: dict = {}


def _build_nc():
    import concourse.bacc as bacc
    import concourse.tile as tile
    from concourse import mybir
    from contextlib import ExitStack

    nc = bacc.Bacc("TRN2", target_bir_lowering=False, debug=False)
    pa01 = nc.dram_tensor("pa01", [P, KPP, 2], mybir.dt.float32, kind="ExternalInput")
    pa23 = nc.dram_tensor("pa23", [P, KPP, 2], mybir.dt.float32, kind="ExternalInput")
    auxS = nc.dram_tensor("auxS", [P, KPP, 2], mybir.dt.float32, kind="ExternalInput")
    auxB = nc.dram_tensor("auxB", [P, KPP, 2], mybir.dt.float32, kind="ExternalInput")
    predsB = nc.dram_tensor("predsB", [P, KPP, 81], mybir.dt.bfloat16, kind="ExternalInput")
    clsc = nc.dram_tensor("clsc", [P, C], mybir.dt.bfloat16, kind="ExternalInput")
    # Lane-plane-per-anchor layout [anchor, lane, class]: every SBUF write
    # that fills it is a contiguous run of 80-320 elements (fast DVE/ACT
    # modes), unlike the [anchor, class, lane] layout whose stride-6 lane
    # scatters ran at 0.3 elem/cycle. Host permutes [6,C]->[C,6] during the
    # bf16->fp32 upconvert pass. Same HBM bytes on device either way.
    out = nc.dram_tensor("out", [P, KPP, 6, C], mybir.dt.bfloat16, kind="ExternalOutput")

    fp32 = mybir.dt.float32
    bf16 = mybir.dt.bfloat16
    AF = mybir.ActivationFunctionType

    with tile.TileContext(nc) as tc, ExitStack() as ctx:
        cpool = ctx.enter_context(tc.tile_pool(name="const", bufs=1))
        ipool = ctx.enter_context(tc.tile_pool(name="in", bufs=NIC))
        opool = ctx.enter_context(tc.tile_pool(name="out", bufs=1))
        tpool = ctx.enter_context(tc.tile_pool(name="tmp", bufs=4))

        pa23_t = cpool.tile([P, KPP, 2], fp32, tag="pa23")
        nc.sync.dma_start(out=pa23_t[:], in_=pa23[:])
        auxS_t = cpool.tile([P, KPP, 2], fp32, tag="auxS")
        nc.sync.dma_start(out=auxS_t[:], in_=auxS[:])
        pa01_t = cpool.tile([P, KPP, 2], fp32, tag="pa01")
        nc.sync.dma_start(out=pa01_t[:], in_=pa01[:])
        auxB_t = cpool.tile([P, KPP, 2], fp32, tag="auxB")
        nc.sync.dma_start(out=auxB_t[:], in_=auxB[:])
        cls_t = cpool.tile([P, C], bf16, tag="cls")
        nc.sync.dma_start(out=cls_t[:], in_=clsc[:])

        # Input chunks: 33 anchors (3 compute tiles) per DMA, ~684 KB each.
        # Chunk 0 rides the Sync HWDGE ring ahead of the big output writes;
        # the rest go on the second HWDGE ring (ACT) so neither GpSimd (now
        # doing the score muls) nor the output FIFO is in their path.
        in_tiles = []
        for ci in range(NIC):
            it = ipool.tile([P, ICH, 81], bf16, tag="pt", name=f"pt{ci}")
            dma_eng = nc.sync if ci == 0 else nc.scalar
            dma_eng.dma_start(out=it[:], in_=predsB[:, ci * ICH : (ci + 1) * ICH, :])
            in_tiles.append(it)

        # Whole-core box decode once, all on contiguous layouts:
        #   bb[:,0] = p_xy * s + grid*s ; bb[:,1] = bb[:,0] + exp(p_wh) * s
        # then interleave into box_t[p, k, 0:4] = (x1, y1, x2, y2) as bf16.
        wh_t = cpool.tile([P, KPP, 2], fp32, tag="wh")
        nc.scalar.activation(wh_t[:], pa23_t[:], AF.Exp)
        nc.vector.tensor_mul(wh_t[:], wh_t[:], auxS_t[:])
        bb_t = cpool.tile([P, 2, KPP, 2], fp32, tag="bb")
        nc.vector.tensor_mul(bb_t[:, 0, :, :], pa01_t[:], auxS_t[:])
        nc.vector.tensor_add(bb_t[:, 0, :, :], bb_t[:, 0, :, :], auxB_t[:])
        nc.vector.tensor_add(bb_t[:, 1, :, :], bb_t[:, 0, :, :], wh_t[:])
        box_t = cpool.tile([P, KPP, 4], bf16, tag="box")
        nc.vector.tensor_copy(
            box_t[:].rearrange("p k (jh jl) -> p k jh jl", jh=2),
            bb_t[:].rearrange("p jh k jl -> p k jh jl"),
        )

        # Persistent out buffers [anchor, lane, class]; constant class-id
        # plane (lane 4) written once per buffer — contiguous 80-elem runs,
        # cheap on DVE.
        ot_bufs = [
            opool.tile([P, GK, 6, C], bf16, tag=f"ot{j}", name=f"ot{j}")
            for j in range(NOB)
        ]
        for j in range(NOB):
            nc.vector.tensor_copy(
                ot_bufs[j][:, :, 4, :],
                cls_t[:].unsqueeze(1).broadcast_to([P, GK, C]),
            )

        # Per chunk: sigmoid of the 80 class logits (contiguous), and
        # sigmoid of the objectness logit pre-broadcast across classes so
        # the per-tile score multiply is an all-step-1 TT (2x DVE mode).
        sig_cls, sig_obj = [], []
        for ci in range(NIC):
            sc = tpool.tile([P, ICH, 80], bf16, tag="sigc", name=f"sigc{ci}")
            nc.scalar.activation(sc[:], in_tiles[ci][:, :, 1:81], AF.Sigmoid)
            sig_cls.append(sc)
            so = tpool.tile([P, ICH, 80], bf16, tag="sigo", name=f"sigo{ci}")
            nc.scalar.activation(
                so[:], in_tiles[ci][:, :, 0:1].broadcast_to([P, ICH, 80]), AF.Sigmoid
            )
            sig_obj.append(so)

        for t in range(NT):
            sl = slice(t * GK, (t + 1) * GK)
            ci = t // 3
            ksl = slice((t % 3) * GK, (t % 3 + 1) * GK)

            ot = ot_bufs[t % NOB]
            nc.vector.tensor_copy(
                ot[:, :, 0:4, :],
                box_t[:, sl, :].unsqueeze(3).broadcast_to([P, GK, 4, C]),
            )
            nc.vector.tensor_mul(
                ot[:, :, 5, :],
                sig_cls[ci][:, ksl, :],
                sig_obj[ci][:, ksl, :],
            )

            nc.sync.dma_start(out=out[:, sl, :, :], in_=ot[:])

    nc.compile()
    return nc


def _host_consts():
    # Per-anchor (stride, stride) and (gx*stride, gy*stride), padded to N_PAD.
    s = np.ones(N_PAD, np.float32)
    bx = np.zeros(N_PAD, np.float32)
    by = np.zeros(N_PAD, np.float32)
    off = 0
    for g, st in ((80, 8.0), (40, 16.0), (20, 32.0)):
        n = g * g
        i = np.arange(n)
        s[off : off + n] = st
        bx[off : off + n] = (i % g) * st
        by[off : off + n] = (i // g) * st
        off += n
    auxS = np.stack([s, s], axis=-1).astype(np.float32)
    auxB = np.stack([bx, by], axis=-1).astype(np.float32)
    auxS = np.concatenate([auxS] * B_PER_CORE, 0).reshape(P, KPP, 2)
    auxB = np.concatenate([auxB] * B_PER_CORE, 0).reshape(P, KPP, 2)
    cls = np.broadcast_to(
        np.arange(C, dtype=np.float32).astype(ml_dtypes.bfloat16), (P, C)
    ).copy()
    return np.ascontiguousarray(auxS), np.ascontiguousarray(auxB), cls


def _host_in_maps(pred0, pred1, pred2):
    auxS, auxB, cls = _CACHE["consts"]
    pred0 = np.asarray(pred0, np.float32).reshape(B, -1, F)
    pred1 = np.asarray(pred1, np.float32).reshape(B, -1, F)
    pred2 = np.asarray(pred2, np.float32).reshape(B, -1, F)
    in_maps = []
    for core in range(N_CORES):
        flat = np.zeros((B_PER_CORE * N_PAD, F), np.float32)
        for j in range(B_PER_CORE):
            b = core * B_PER_CORE + j
            flat[j * N_PAD : j * N_PAD + N_REAL] = np.concatenate(
                [pred0[b], pred1[b], pred2[b]], axis=0
            )
        in_maps.append(
            {
                "pa01": np.ascontiguousarray(flat[:, 0:2]).reshape(P, KPP, 2),
                "pa23": np.ascontiguousarray(flat[:, 2:4]).reshape(P, KPP, 2),
                "auxS": auxS,
                "auxB": auxB,
                "predsB": np.ascontiguousarray(flat[:, 4:85]).astype(ml_dtypes.bfloat16).reshape(P, KPP, 81),
                "clsc": cls,
            }
        )
    return in_maps


def kernel(images, pred0, pred1, pred2):
    from concourse.bass_utils import run_bass_kernel_spmd

    if "nc" not in _CACHE:
        _CACHE["consts"] = _host_consts()
        _CACHE["nc"] = _build_nc()
    nc = _CACHE["nc"]

    in_maps = _host_in_maps(pred0, pred1, pred2)
    res = run_bass_kernel_spmd(nc, in_maps, list(range(N_CORES)))
    final = np.empty((B, N_REAL * C, 6), np.float32)
    for core, r in enumerate(res.results):
        # Device layout is [anchor, lane, C]; swap to [anchor, C, lane] while
        # upconverting bf16 -> fp32 in one strided assign.
        src = r["out"].reshape(B_PER_CORE, N_PAD, 6, C)[:, :N_REAL].transpose(
            0, 1, 3, 2
        )
        final[core * B_PER_CORE : (core + 1) * B_PER_CORE].reshape(
            B_PER_CORE, N_REAL, C, 6
        )[:] = src
    return final
